# revision 36
# baseline (speedup 1.0000x reference)
"""BlockSparse Ring Multihead Dilated Attention — Trainium2 Bass kernel.

Decomposition: the LongNet-style dilated attention factors into 28 independent
dense 2048x2048 attention "units" (one per head x segment x dilation-offset).
Each of the 8 cores gets a perfectly balanced bundle:
  set A: one group-0 pair   (2 heads, same 2048-token segment, 2048 queries)
  set B: one group-1 pair   (2 heads, same dilated 2048-token set, 1024 queries)
  set C: one group-2 head   (1 head, dilated 2048-token set, 1024 queries)

On-device structure (per core), engineered around the TRN2 cost model
(matmul cost = out-free-rows x cycles/row; ACT exp is a ~100us floor):
  - QKV projections in bf16 (6-chunk contraction over the embed dim).
  - Scores S^T = K^T Q in bf16 -> PSUM, exp on ACT writes P in bf16 to SBUF.
  - AV as O[q-part, d-free]: lhsT = P block [128k, 128q], rhs = V [128k, 65]
    (ones column appended for the softmax denominator) — full 128 output
    partitions, 65 rows per matmul instead of 512.
  - Normalization: per-partition reciprocal-scalar multiply (DVE).
  - O transposed to [d, q] via DMA XBAR transpose, out-proj in bf16,
    PSUM drained by DVE+Pool, DMA to DRAM.
  - Proj of the next set and out-proj of the previous q-chunk are interleaved
    into the exp-bound inner loop via a pending-work queue.
The host scatter-adds the 8 partial out-projections and adds the output bias
(including the folded V-bias term  bv_h @ Wo_h summed over heads).
"""

from collections import deque

import numpy as np
import ml_dtypes

BF16 = ml_dtypes.bfloat16

SETS = ("A", "B", "C")
NH = {"A": 2, "B": 2, "C": 1}      # heads per set
NQ = {"A": 2048, "B": 1024, "C": 1024}  # queries per set
ROW0 = {"A": 0, "B": 2048, "C": 3072}   # zout row offset


def _w_offsets():
    off, c = {}, 0
    for s in SETS:
        m = 64 * NH[s]
        off[s] = {}
        if s == "C":
            off[s]["qk"] = c          # packed [Wq; Wk] -> 128 rows
            c += 6 * 128
            off[s]["v"] = c
            c += 6 * m
        else:
            for nm in ("q", "k", "v"):
                off[s][nm] = c
                c += 6 * m
    return off, c


_WOFF, _WCOLS = _w_offsets()  # 5760
_BCOL = {"A": {"q": 0, "k": 1}, "B": {"q": 2, "k": 3}, "C": {"q": 4, "k": 5}}

_CACHE = {}


def _core_plan(c):
    """Unit assignment for core c (0..7)."""
    # set A: group-0 (seg 2048, r=1): seg = c//2, heads (0,1) or (2,3)
    segA = c // 2
    haA = 2 * (c % 2)
    orderA = segA * 2048 + np.arange(2048)
    # set B: group-1 (seg 4096, r=2): pairs (4,6) parity0 / (5,7) parity1
    seg1 = c // 4
    p = (c % 4) // 2
    qh_b = c % 2
    hB = (4 + p, 6 + p)
    tokB = seg1 * 4096 + p + 2 * np.arange(2048)
    orderB = np.concatenate([tokB[qh_b * 1024:(qh_b + 1) * 1024],
                             tokB[(1 - qh_b) * 1024:(2 - qh_b) * 1024]])
    # set C: group-2 (seg 8192, r=4): head 8+j owns tokens j + 4*arange
    j = c // 2
    qh_c = c % 2
    hC = 8 + j
    tokC = j + 4 * np.arange(2048)
    orderC = np.concatenate([tokC[qh_c * 1024:(qh_c + 1) * 1024],
                             tokC[(1 - qh_c) * 1024:(2 - qh_c) * 1024]])
    return {
        "A": {"heads": (haA, haA + 1), "order": orderA},
        "B": {"heads": hB, "order": orderB},
        "C": {"heads": (hC,), "order": orderC},
    }


def _pack_lhsT(wrows):
    """[M, 768] weight rows -> bf16 [128, 6*M] (e-chunked lhsT layout)."""
    m = wrows.shape[0]
    t = wrows.T.reshape(6, 128, m).transpose(1, 0, 2)
    return np.ascontiguousarray(t.reshape(128, 6 * m)).astype(BF16)


def _prep_core_inputs(c, x, qkv_w, qkv_b, out_w):
    plan = _core_plan(c)
    x2 = x[0]  # [8192, 768] f32
    ins = {}
    wcols = []
    wo = np.zeros((128, 3, 768), np.float32)
    bqk = np.zeros((128, 6), np.float32)
    for si, s in enumerate(SETS):
        heads = plan[s]["heads"]
        order = plan[s]["order"]
        m = 64 * NH[s]
        xs = x2[order]  # [2048, 768]
        xt = xs.T.reshape(6, 128, 2048).transpose(1, 0, 2)
        ins[f"xt{s}"] = np.ascontiguousarray(xt).astype(BF16)
        qrows = np.concatenate([qkv_w[h * 64:(h + 1) * 64] for h in heads], 0)
        krows = np.concatenate([qkv_w[768 + h * 64:768 + (h + 1) * 64] for h in heads], 0)
        vrows = np.concatenate([qkv_w[1536 + h * 64:1536 + (h + 1) * 64] for h in heads], 0)
        if s == "C":
            wcols += [_pack_lhsT(np.concatenate([qrows, krows], 0)),
                      _pack_lhsT(vrows)]
        else:
            wcols += [_pack_lhsT(qrows), _pack_lhsT(krows), _pack_lhsT(vrows)]
        wos = np.concatenate([out_w[:, h * 64:(h + 1) * 64].T for h in heads], 0)
        wo[:m, si] = wos
        bqk[:m, _BCOL[s]["q"]] = np.concatenate(
            [qkv_b[h * 64:(h + 1) * 64] for h in heads])
        bqk[:m, _BCOL[s]["k"]] = np.concatenate(
            [qkv_b[768 + h * 64:768 + (h + 1) * 64] for h in heads])
        # cols 4/5 double as the packed-C q/k biases, both at base 0
    ins["w16"] = np.concatenate(wcols, axis=1)
    assert ins["w16"].shape == (128, _WCOLS), ins["w16"].shape
    ins["wo16"] = wo.astype(BF16)
    ins["bqk"] = bqk
    return ins


def _build_module():
    from concourse import bacc
    import concourse.mybir as mybir
    import concourse.tile as tile
    from concourse.bass import ts, ds

    dt = mybir.dt
    f32, bf = dt.float32, dt.bfloat16
    EXP = mybir.ActivationFunctionType.Exp
    MULT = mybir.AluOpType.mult

    nc = bacc.Bacc("TRN2", target_bir_lowering=False, debug=False)

    xtd = {s: nc.dram_tensor(f"xt{s}", (128, 6, 2048), bf, kind="ExternalInput")
           for s in SETS}
    wd = nc.dram_tensor("w16", (128, _WCOLS), bf, kind="ExternalInput")
    wod = nc.dram_tensor("wo16", (128, 3, 768), bf, kind="ExternalInput")
    bqkd = nc.dram_tensor("bqk", (128, 6), f32, kind="ExternalInput")
    zoutd = nc.dram_tensor("zout", (4096, 768), f32, kind="ExternalOutput")

    with tile.TileContext(nc) as tc:
        with (
            tc.tile_pool(name="const", bufs=1) as constp,
            tc.tile_pool(name="ptp", bufs=2) as ptpool,
            tc.tile_pool(name="osbp", bufs=3) as osbp,
            tc.tile_pool(name="rsbp", bufs=2) as rsbp,
            tc.tile_pool(name="zsbp", bufs=4) as zsbp,
            tc.tile_pool(name="pbig", bufs=1, space="PSUM") as pbig,
            tc.tile_pool(name="po", bufs=1, space="PSUM") as po,
        ):
            wsb = constp.tile([128, _WCOLS], bf, name="wsb")
            wosb = constp.tile([128, 3, 768], bf, name="wosb")
            bsb = constp.tile([128, 6], f32, name="bsb")
            xts = {s: constp.tile([128, 6, 2048], bf, name=f"xt{s}")
                   for s in SETS}

            def dma_w(s, nm):
                m6 = 6 * 128 if nm == "qk" else 6 * 64 * NH[s]
                off = _WOFF[s][nm]
                nc.sync.dma_start(wsb[:, ds(off, m6)], wd[:, ds(off, m6)])

            def dma_xt(s, qt, split=False):
                if split:
                    for eh in range(3):
                        nc.sync.dma_start(
                            xts[s][:, ds(eh * 2, 2), ds(qt * 512, 512)],
                            xtd[s][:, ds(eh * 2, 2), ds(qt * 512, 512)])
                else:
                    nc.sync.dma_start(xts[s][:, :, ds(qt * 512, 512)],
                                      xtd[s][:, :, ds(qt * 512, 512)])

            # critical path first: set-A k/q weights + xtA, then the rest.
            nc.sync.dma_start(bsb[:], bqkd[:])
            dma_w("A", "k")
            dma_w("A", "q")
            dma_xt("A", 0, split=True)
            for qt in range(1, 4):
                dma_xt("A", qt, split=True)
            dma_w("A", "v")
            for s in ("B", "C"):
                for qt in range(4):
                    dma_xt(s, qt)
                for nm in (("k", "q", "v") if s != "C" else ("qk", "v")):
                    dma_w(s, nm)
            nc.sync.dma_start(wosb[:], wod[:])
            qTs = {"A": constp.tile([128, 2048], bf, name="qTA"),
                   "B": constp.tile([128, 1024], bf, name="qTB"),
                   "C": constp.tile([64, 1024], bf, name="qTC")}
            kTs = {"A": constp.tile([128, 2048], bf, name="kTA"),
                   "B": constp.tile([128, 2048], bf, name="kTB"),
                   "C": constp.tile([64, 2048], bf, name="kTC")}
            v16s = {s: constp.tile([128, 8, 2, 65 * NH[s]], bf, name=f"v16{s}")
                    for s in SETS}

            ntag = [0]

            def qk_half(s, nm, hf):
                m = 64 * NH[s]
                dst = qTs[s] if nm == "q" else kTs[s]
                bcol = _BCOL[s][nm]
                woff = _WOFF[s][nm]
                tg = "abc"[ntag[0] % 3]
                ntag[0] += 1

                def piece():
                    ps = pbig.tile([128, 2, 512], f32, tag=tg,
                                   name=f"ph{s}{nm}{hf}")
                    for e in range(6):
                        nc.tensor.matmul(
                            ps[:m, 0],
                            lhsT=wsb[:, ds(woff + e * m, m)],
                            rhs=xts[s][:, e, ts(hf, 512)],
                            start=(e == 0), stop=(e == 5))
                    nc.vector.tensor_scalar_add(
                        dst[:m, ds(512 * hf, 512)],
                        ps[:m, 0],
                        bsb[:m, bcol:bcol + 1])
                return piece

            def qk_packed_c(bt):
                # C: one [q64|k64] 128-wide chain over 1024 tokens; the
                # unused q half of bt1 rides free in the same matmuls.
                woff = _WOFF["C"]["qk"]
                tg = "abc"[ntag[0] % 3]
                ntag[0] += 1

                def piece():
                    ps = pbig.tile([128, 2, 512], f32, tag=tg,
                                   name=f"pqk{bt}")
                    for half in range(2):
                        for e in range(6):
                            nc.tensor.matmul(
                                ps[:, half],
                                lhsT=wsb[:, ds(woff + e * 128, 128)],
                                rhs=xts["C"][:, e, ts(2 * bt + half, 512)],
                                start=(e == 0), stop=(e == 5))
                    if bt == 0:
                        nc.vector.tensor_scalar_add(
                            qTs["C"][0:64, 0:1024],
                            ps[0:64].rearrange("p a b -> p (a b)"),
                            bsb[0:64, 4:5])
                    nc.vector.tensor_scalar_add(
                        kTs["C"][0:64, ds(1024 * bt, 1024)],
                        ps[64:128].rearrange("p a b -> p (a b)"),
                        bsb[0:64, 5:6])
                return piece

            def qk_piece(s, nm, bt):
                m = 64 * NH[s]
                dst = qTs[s] if nm == "q" else kTs[s]
                bcol = _BCOL[s][nm]
                woff = _WOFF[s][nm]
                tg = "abc"[ntag[0] % 3]
                ntag[0] += 1

                def piece():
                    ps = pbig.tile([128, 2, 512], f32, tag=tg,
                                   name=f"ps{s}{nm}{bt}")
                    for half in range(2):
                        for e in range(6):
                            nc.tensor.matmul(
                                ps[:m, half],
                                lhsT=wsb[:, ds(woff + e * m, m)],
                                rhs=xts[s][:, e, ts(2 * bt + half, 512)],
                                start=(e == 0), stop=(e == 5))
                    nc.vector.tensor_scalar_add(
                        dst[:m, ds(1024 * bt, 1024)],
                        ps[:m].rearrange("p a b -> p (a b)"),
                        bsb[:m, bcol:bcol + 1])
                return piece

            def v_piece(s, vt, nslots):
                m = 64 * NH[s]
                woff = _WOFF[s]["v"]
                tg = "abc"[ntag[0] % 3]
                ntag[0] += 1
                per = 512 // m  # slots per 512-col half

                nh = NH[s]

                def piece():
                    ps = pbig.tile([128, 2, 512], f32, tag=tg,
                                   name=f"psv{s}{vt}")
                    for i in range(nslots):
                        tt = vt * nslots + i
                        slot = ps[:, i // per, ts(i % per, m)]
                        # start zeroes the whole 2KB bank: only the first
                        # slot of each bank-half starts; later slots rely on
                        # the pending-zero fresh-write.
                        for e in range(6):
                            nc.tensor.matmul(
                                slot, lhsT=xts[s][:, e, ts(tt, 128)],
                                rhs=wsb[:, ds(woff + e * m, m)],
                                start=(e == 0 and i % per == 0),
                                stop=(e == 5 and (i + 1) % per == 0),
                                skip_group_check=True)
                    for half in range(2):
                        kp0 = (vt * nslots + half * per) // 2
                        nkp = per // 2
                        vdst = v16s[s][:, ds(kp0, nkp)].rearrange(
                            "p a b (h w) -> p a b h w", w=65)[:, :, :, :, 0:64]
                        src = ps[:, half].rearrange(
                            "p (a b h d) -> p a b h d", b=2, h=nh, d=64)
                        nc.vector.tensor_copy(vdst, src)
                return piece

            def proj_pieces(s):
                out = []
                vv = v16s[s].rearrange("p a b (h w) -> p a b h w", w=65)

                def ones():
                    nc.gpsimd.memset(vv[:, :, :, :, 64:65], 1.0)
                out.append(ones)
                if s == "C":
                    out.append(qk_packed_c(0))
                    out.append(qk_packed_c(1))
                else:
                    for bt in range(2):
                        out.append(qk_piece(s, "k", bt))
                    out.append(qk_piece(s, "q", 0))
                nslots = 8 if NH[s] == 2 else 16
                for vt in range(16 // nslots):
                    out.append(v_piece(s, vt, nslots))
                for bt in range(1, NQ[s] // 1024):
                    if s != "C":
                        out.append(qk_piece(s, "q", bt))
                return out

            pending = deque()        # proj pieces — safe to run any slot
            pending_slow = deque()   # outproj pieces — gated to ktp >= 2

            def pump(n, slow_ok=True):
                for _ in range(n):
                    if pending:
                        pending.popleft()()
                    elif slow_ok and pending_slow:
                        pending_slow.popleft()()

            COPY = mybir.ActivationFunctionType.Copy

            def outproj_piece(s, si, mc, tc, osb, zsb, tail=False):
                m = 64 * NH[s]

                def piece():
                    tg = "abc"[ntag[0] % 3]
                    ntag[0] += 1
                    zt = pbig.tile([128, 2, 512], f32, tag=tg,
                                   name=f"z{s}{mc}{tc}")
                    za = zt[:, 0]
                    zb = zt[:, 1, 0:256]
                    nc.tensor.matmul(za, lhsT=osb[0:m, ts(tc, 128)],
                                     rhs=wosb[0:m, si, 0:512],
                                     start=True, stop=True)
                    nc.tensor.matmul(zb, lhsT=osb[0:m, ts(tc, 128)],
                                     rhs=wosb[0:m, si, 512:768],
                                     start=True, stop=True)
                    if tail:
                        # ACT is idle at the tail; split the PSUM drain
                        nc.vector.tensor_copy(zsb[:, 0:384], za[:, 0:384])
                        nc.scalar.activation(zsb[:, 384:512], za[:, 384:512],
                                             COPY)
                        nc.scalar.activation(zsb[:, 512:768], zb, COPY)
                    else:
                        nc.vector.tensor_copy(zsb[:, 0:512], za)
                        nc.vector.tensor_copy(zsb[:, 512:768], zb)
                    nc.sync.dma_start(
                        zoutd[ds(ROW0[s] + mc * 512 + tc * 128, 128), :],
                        zsb[:])
                return piece

            def emit_attn_set(s, si):
                hoffs = [(0, 0)] + ([(1, 64)] if NH[s] == 2 else [])
                for mc in range(NQ[s] // 512):
                    o_t = {}
                    for hi, _ in hoffs:
                        o_t[hi] = po.tile([65, 512], f32, tag=f"o{'ab'[hi]}",
                                          name=f"o{s}{mc}{hi}")
                    # two-deep software pipeline: S(k) | exp(k-1) | AV(k-2)
                    # — every cross-engine dependency is ~2us stale by the
                    # time its consumer dispatches, hiding semaphore
                    # propagation latency.
                    def emit_av(ktp, hi, pt):
                        for jj in range(2):
                            kt = 2 * ktp + jj
                            nc.tensor.matmul(
                                o_t[hi][:],
                                lhsT=v16s[s][:, ktp, jj, ds(65 * hi, 65)],
                                rhs=pt[:, jj, :],
                                start=(kt == 0), stop=(kt == 15),
                                skip_group_check=True)

                    def emit_exp(sts):
                        for st, pt in sts:
                            nc.scalar.activation(pt[:], st[:], EXP, scale=0.125)

                    stage_s = deque()   # (ktp, [(st, pt)...])
                    stage_e = deque()   # (ktp, [(hi, pt)...])
                    for ktp in range(8):
                        sts = []
                        avs = []
                        for hi, hoff in hoffs:
                            tg = "abc"[ntag[0] % 3]
                            ntag[0] += 1
                            st = pbig.tile([128, 2, 512], f32, tag=tg,
                                           name=f"st{s}{mc}{ktp}{hi}")
                            for jj in range(2):
                                kt = 2 * ktp + jj
                                nc.tensor.matmul(
                                    st[:, jj],
                                    lhsT=kTs[s][hoff:hoff + 64, ts(kt, 128)],
                                    rhs=qTs[s][hoff:hoff + 64, ts(mc, 512)],
                                    start=True, stop=True)
                            pt = ptpool.tile([128, 2, 512], bf, tag=f"pt{tg}",
                                             name=f"pt{s}{mc}{ktp}{hi}")
                            sts.append((st, pt))
                            avs.append((ktp, hi, pt))
                        stage_s.append(sts)
                        stage_e.append(avs)
                        if len(stage_s) >= 2:
                            emit_exp(stage_s.popleft())
                        # single-head set C: ktps are ~2x shorter, so one
                        # extra ktp of AV lag is needed to cover the norm
                        # chain at mc boundaries.
                        av_lag = 4
                        if len(stage_e) >= av_lag:
                            for args in stage_e.popleft():
                                emit_av(*args)
                        # outproj pieces are gated to ktp >= 2: earlier they
                        # would block the in-order PE queue on the norm of the
                        # previous mc. During the final mc (C, mc1) they are
                        # reserved entirely for the flush, where they keep PE
                        # fed while the last norm chain completes.
                        last_mc = (s == "C" and mc == 1)
                        pump(1 if s == "A" else 2,
                             slow_ok=(ktp >= 2 and not last_mc))
                    while stage_s:
                        emit_exp(stage_s.popleft())
                    while stage_e:
                        for args in stage_e.popleft():
                            emit_av(*args)
                    # normalize: O = O'[0:64] * (1/D); D = O'[64]. Fast
                    # approx reciprocal (1 DVE op), Pool broadcasts it across
                    # partitions, one fused multiply drains PSUM -> osb
                    # ([d, tok] bf16 — directly the out-proj lhsT layout).
                    osb = osbp.tile([128, 512], bf, tag="osb",
                                    name=f"osb{s}{mc}")
                    dsb = rsbp.tile([1, 2, 512], f32, tag="dsb",
                                    name=f"dsb{s}{mc}")
                    rsb = rsbp.tile([1, 2, 512], f32, tag="rsb",
                                    name=f"rsb{s}{mc}")
                    rbts = {hi: rsbp.tile([64, 512], f32, tag=f"rbt{hi}",
                                          name=f"rbt{s}{mc}{hi}")
                            for hi, _ in hoffs}
                    for hi, hoff in hoffs:
                        # regular copy handles the partition-base-64 PSUM
                        # read; custom DVE ops require base-0 operands.
                        nc.vector.tensor_copy(dsb[0:1, hi, :],
                                              o_t[hi][64:65, :])
                        nc.vector.reciprocal_approx_fast(
                            rsb[0:1, hi, :], dsb[0:1, hi, :])
                        nc.gpsimd.partition_broadcast(
                            rbts[hi][:], rsb[0:1, hi, :])
                        nc.vector.tensor_tensor(
                            osb[hoff:hoff + 64, :], o_t[hi][0:64, :],
                            rbts[hi][:], MULT)
                    for tc in range(4):
                        zsb = zsbp.tile([128, 768], f32, tag="zsb",
                                        name=f"zsb{s}{mc}{tc}")
                        pending_slow.append(
                            outproj_piece(s, si, mc, tc, osb, zsb,
                                          tail=(s == "C" and mc == 1)))

            # minimal set-A critical path up front (q cols 0:512 + all k);
            # everything else streams through the pending queue inside the
            # exp-bound attention inner loop.
            vvA = v16s["A"].rearrange("p a b (h w) -> p a b h w", w=65)
            nc.gpsimd.memset(vvA[:, :, :, :, 64:65], 1.0)
            qk_half("A", "q", 0)()
            qk_half("A", "k", 0)()
            pending.append(qk_half("A", "k", 1))
            pending.append(v_piece("A", 0, 8))
            pending.append(qk_half("A", "k", 2))
            pending.append(v_piece("A", 1, 8))
            pending.append(qk_half("A", "k", 3))
            pending.append(qk_half("A", "q", 1))
            pending.append(qk_piece("A", "q", 1))
            pending.extend(proj_pieces("B"))
            pending.extend(proj_pieces("C"))
            emit_attn_set("A", 0)
            emit_attn_set("B", 1)
            emit_attn_set("C", 2)
            while pending or pending_slow:
                pump(len(pending) + len(pending_slow))

    nc.compile()
    return nc


def _get_module():
    if "nc" not in _CACHE:
        _CACHE["nc"] = _build_module()
    return _CACHE["nc"]


def _assemble(results, zbias):
    out = np.zeros((8192, 768), np.float32)
    for c in range(8):
        plan = _core_plan(c)
        z = results[c]["zout"]
        out[plan["A"]["order"]] += z[0:2048]
        out[plan["B"]["order"][:1024]] += z[2048:3072]
        out[plan["C"]["order"][:1024]] += z[3072:4096]
    out += zbias[None, :]
    return out.reshape(1, 8192, 768)


def kernel(x, qkv_w, qkv_b, out_w, out_b, _trace=False):
    x = np.asarray(x, np.float32)
    qkv_w = np.asarray(qkv_w, np.float32)
    qkv_b = np.asarray(qkv_b, np.float32)
    out_w = np.asarray(out_w, np.float32)
    out_b = np.asarray(out_b, np.float32)

    from concourse.bass_utils import run_bass_kernel_spmd

    nc = _get_module()
    in_maps = [_prep_core_inputs(c, x, qkv_w, qkv_b, out_w) for c in range(8)]
    # fold the V-bias through the output projection:  (PV/D + bv) Wo
    zbias = out_b.copy()
    for h in range(12):
        zbias += qkv_b[1536 + h * 64:1536 + (h + 1) * 64] @ \
            out_w[:, h * 64:(h + 1) * 64].T
    res = run_bass_kernel_spmd(nc, in_maps, core_ids=list(range(8)), trace=_trace)
    out = _assemble(res.results, zbias)
    if _trace:
        _CACHE["last_result"] = res
    return out
